# revision 1
# baseline (speedup 1.0000x reference)
"""Trainium2 Bass kernel for: out = A @ dequant_int4(weight, weight_scale) + bias.

Problem shapes (fp32 A, packed-int4 weight):
    A            [8192, 4096] f32
    weight       [2048, 11008] u8   (two int4 nibbles per byte along K;
                                     row 2i = low nibble, row 2i+1 = high nibble)
    weight_scale [128, 11008] f32   (per-group scale, group_size=32 along K)
    bias         [11008] f32
    out          [8192, 11008] f32

Sharding: tensor-parallel along out_features N across 8 NeuronCores.
Each core gets the full A, a 1376-wide column slice of weight/scale/bias and
computes its [8192, 1376] output slice; the host concatenates slices.

Per-core kernel strategy (v3 "hostT" — measured ~1.4-1.6 ms/core vs 4.2 ms
baseline; PE roofline for the 92 GFLOP slice is ~1.31 ms):
  - A is transposed AND cast to bf16 on the host into wsb's
    nibble-interleaved k-order, so the device does zero transposes: no PE
    transpose instructions, no PSUM->SBUF copies, and A's HBM traffic
    halves.  Each 128-row chunk is one clean [128, 4096] bf16 HWDGE load
    on the scalar ring (prefetched 3 deep), while OUT stores ride the sync
    ring.
  - The int4 weight slice is dequantized once into resident SBUF bf16
    (~86 KB/partition), as three per-n-chunk tiles emitted n-chunk-major;
    Tile's range-precise dependency tracking lets the first matmuls start
    after the first 128-row dequant block, hiding most of the dequant head.
  - Matmul loop is n-chunk-outer (long same-PSUM-bank accumulation runs —
    interleaving anything between accumulating matmuls measurably stalls
    the PE), with psum_o rotated over all 8 PSUM banks so a chunk's first
    matmul never waits on a bias-add eviction (DVE) of an older bank.
"""

import numpy as np

import concourse.bacc as bacc
import concourse.bass as bass
import concourse.tile as tile
from concourse import mybir
from concourse.bass_utils import run_bass_kernel_spmd
from concourse.masks import make_identity

M, K, N = 8192, 4096, 11008
NCORES = 8
NS = N // NCORES  # 1376 out-features per core
K2 = K // 2       # 2048 packed rows
P = 128
NB2 = K2 // P     # 16 packed k-blocks
NKB = K // P      # 32 unpacked k-blocks


class _WrapSlice:
    """Row-slice wrapper that wraps row indices modulo a small DRAM tensor.

    Lets timing builds keep the exact per-chunk DMA shapes while the backing
    DRAM tensor only has `rows` rows.  Only row slices with step P-aligned
    to `rows` are used by the kernel."""

    def __init__(self, t, rows):
        self._t = t
        self._rows = rows

    def __getitem__(self, idx):
        rs, cs = idx
        start = rs.start % self._rows
        stop = start + (rs.stop - rs.start)
        assert stop <= self._rows, (rs, self._rows)
        return self._t[start:stop, cs]


def _n_chunks(ns, step=512):
    out = []
    n0 = 0
    while n0 < ns:
        out.append((n0, min(step, ns - n0)))
        n0 += step
    return out


def build_nc(m=M, ns=NS, debug=False, reps=1, variant="full", timing_io=False):
    """Build the per-core Bass program (identical on all cores).

    reps > 1 repeats the whole body (for slope-based HW timing).
    timing_io=True shrinks the A/OUT DRAM tensors (device work unchanged)
    so the axon per-call transfer cost doesn't swamp slope timing.
    variant: "full" (correct kernel) or a timing-probe variant:
      "no_tr"      - at tiles written once by DMA; no transposes/copies
      "tr_no_copy" - transposes run (PE cost) but copies skipped; at from DMA
      "tail_tr"    - transposes clustered after each chunk's matmuls
    """
    mch = m // P
    n_chunks = _n_chunks(ns)
    base_variant = variant.rstrip("4678")

    # Bacc (not raw Bass): its compile() legalizes multi-semaphore waits into
    # the single event slot each DMA/engine instruction has in the ISA.
    nc = bacc.Bacc(None, target_bir_lowering=False, debug=debug)
    a_rows = 2 * P if timing_io else m
    out_rows = P if timing_io else m
    if variant.startswith("hostT"):
        # A arrives host-transposed+cast: AT[mc*128+p, kb*128+j] =
        # A[mc*128+j, kb*128+p], bf16.
        A = nc.dram_tensor("at", [a_rows, K], mybir.dt.bfloat16,
                           kind="ExternalInput")
    else:
        A = nc.dram_tensor("A", [a_rows, K], mybir.dt.float32,
                           kind="ExternalInput")
    WQ = nc.dram_tensor("wq", [K2, ns], mybir.dt.uint8, kind="ExternalInput")
    SREP = nc.dram_tensor("srep", [K2, ns], mybir.dt.float32, kind="ExternalInput")
    BIAS = nc.dram_tensor("bias", [P, ns], mybir.dt.float32, kind="ExternalInput")
    OUT = nc.dram_tensor("out", [out_rows, ns], mybir.dt.float32,
                         kind="ExternalOutput")
    if timing_io:
        A = _WrapSlice(A, 2 * P)
        OUT = _WrapSlice(OUT, P)

    with tile.TileContext(nc) as tc:
        with (
            tc.tile_pool(name="singles", bufs=1) as singles,
            tc.tile_pool(name="wpool", bufs=1) as wpool,
            tc.tile_pool(name="dq", bufs=3) as dq,
            tc.tile_pool(name="apool", bufs=3) as apool,
            tc.tile_pool(name="atpool", bufs=2 * NKB) as atpool,
            tc.tile_pool(name="opool", bufs=3) as opool,
            tc.tile_pool(name="psum_t", bufs=4, space="PSUM") as psum_t,
            tc.tile_pool(
                name="psum_o",
                bufs=(4 if variant.endswith("4") else
                      6 if variant.endswith("6") else
                      7 if variant.endswith("7") else
                      8 if variant.endswith("8") else 3),
                space="PSUM") as psum_o,
        ):
            identity = singles.tile([P, P], mybir.dt.bfloat16)
            make_identity(nc, identity)

            # bias arrives host-replicated to [P, ns]: a stride-0 broadcast DMA
            # trips walrus codegen ("Too many sync wait commands")
            bias_t = singles.tile([P, ns], mybir.dt.float32)
            nc.sync.dma_start(out=bias_t, in_=BIAS[:, :])

            for _rep in range(reps):
                _build_body(nc, wpool, dq, apool, atpool, opool, psum_t, psum_o,
                            identity, bias_t, A, WQ, SREP, OUT, mch, n_chunks,
                            base_variant)

    nc.finalize()
    return nc


def _dequant_chunk(nc, dq, w_t, WQ, SREP, n0, nch):
    for b in range(NB2):
        pk = dq.tile([P, 512], mybir.dt.uint8, tag="pk")
        nc.sync.dma_start(out=pk[:, :nch], in_=WQ[b * P:(b + 1) * P, n0:n0 + nch])
        st = dq.tile([P, 512], mybir.dt.float32, tag="st")
        nc.sync.dma_start(out=st[:, :nch], in_=SREP[b * P:(b + 1) * P, n0:n0 + nch])
        # walrus requires each tensor_scalar's ops to be a single
        # ISA-supported class: bitwise extract (u8->u8), then an
        # arithmetic subtract with the int->float cast on output.
        lo = dq.tile([P, 512], mybir.dt.bfloat16, tag="lo")
        hi = dq.tile([P, 512], mybir.dt.bfloat16, tag="hi")
        lq = dq.tile([P, 512], mybir.dt.uint8, tag="lq")
        hq = dq.tile([P, 512], mybir.dt.uint8, tag="hq")
        nc.vector.tensor_scalar(
            out=lq[:, :nch], in0=pk[:, :nch], scalar1=15, scalar2=None,
            op0=mybir.AluOpType.bitwise_and)
        nc.vector.tensor_scalar(
            out=hq[:, :nch], in0=pk[:, :nch], scalar1=4, scalar2=None,
            op0=mybir.AluOpType.logical_shift_right)
        nc.vector.tensor_scalar(
            out=lo[:, :nch], in0=lq[:, :nch], scalar1=8, scalar2=None,
            op0=mybir.AluOpType.subtract)
        nc.vector.tensor_scalar(
            out=hi[:, :nch], in0=hq[:, :nch], scalar1=8, scalar2=None,
            op0=mybir.AluOpType.subtract)
        nc.vector.tensor_tensor(
            out=w_t[:, 2 * b, :], in0=lo[:, :nch], in1=st[:, :nch],
            op=mybir.AluOpType.mult)
        nc.vector.tensor_tensor(
            out=w_t[:, 2 * b + 1, :], in0=hi[:, :nch], in1=st[:, :nch],
            op=mybir.AluOpType.mult)


def _build_body(nc, wpool, dq, apool, atpool, opool, psum_t, psum_o,
                identity, bias_t, A, WQ, SREP, OUT, mch, n_chunks,
                variant="full"):
    # ---- one-shot dequant of the weight slice into resident SBUF ----
    # One tile per n-chunk, emitted n-chunk-major, so chunk-0 matmuls can
    # begin while later n-chunks are still dequantizing.
    # ---- phase-isolation probes ----
    if variant == "acast_only":
        for mc in range(mch):
            a_nat = apool.tile([P, K], mybir.dt.bfloat16)
            nc.gpsimd.dma_start(out=a_nat, in_=A[(mc % 2) * P:(mc % 2 + 1) * P, :])
        o_tok = opool.tile([P, 16], mybir.dt.float32)
        nc.vector.tensor_scalar(out=o_tok, in0=a_nat[:, :16], scalar1=0,
                                scalar2=None, op0=mybir.AluOpType.add)
        nc.sync.dma_start(out=OUT[0:P, 0:16], in_=o_tok)
        return

    wsb = []
    if variant == "dq_only":
        for ci, (n0, nch) in enumerate(n_chunks):
            w_t = wpool.tile([P, NKB, nch], mybir.dt.bfloat16, tag=f"w{n0}")
            wsb.append(w_t)
            _dequant_chunk(nc, dq, w_t, WQ, SREP, n0, nch)
        o_tok = opool.tile([P, 16], mybir.dt.float32)
        nc.vector.tensor_scalar(out=o_tok, in0=wsb[0][:, 0, :16], scalar1=0,
                                scalar2=None, op0=mybir.AluOpType.add)
        nc.sync.dma_start(out=OUT[0:P, 0:16], in_=o_tok)
        return

    if variant in ("no_dq", "lean_mm", "lean_half"):
        # timing probe: fill wsb by cast-DMA only (no DVE dequant math)
        for ci, (n0, nch) in enumerate(n_chunks):
            w_t = wpool.tile([P, NKB, nch], mybir.dt.bfloat16, tag=f"w{n0}")
            wsb.append(w_t)
            for kb in range(NKB):
                b = kb // 2
                nc.gpsimd.dma_start(
                    out=w_t[:, kb, :],
                    in_=SREP[b * P:(b + 1) * P, n0:n0 + nch])
        n_chunks_iter = []
    else:
        n_chunks_iter = n_chunks
    for (n0, nch) in n_chunks_iter:
        w_t = wpool.tile([P, NKB, nch], mybir.dt.bfloat16, tag=f"w{n0}")
        wsb.append(w_t)
        _dequant_chunk(nc, dq, w_t, WQ, SREP, n0, nch)

    # ---- hostT: A arrives pre-transposed/cast; pure matmul pipeline ----
    if variant == "hostT":
        at_bufs = {}

        def dma_at(mc):
            at_t = apool.tile([P, K], mybir.dt.bfloat16, tag="at")
            nc.scalar.dma_start(out=at_t, in_=A[mc * P:(mc + 1) * P, :])
            return at_t

        for mc in range(min(3, mch)):
            at_bufs[mc] = dma_at(mc)
        for mc in range(mch):
            if mc + 3 < mch:
                at_bufs[mc + 3] = dma_at(mc + 3)
            at_t = at_bufs.pop(mc)
            o_sb = opool.tile([P, sum(c for _, c in n_chunks)], mybir.dt.float32)
            for ci, (n0, nch) in enumerate(n_chunks):
                po = psum_o.tile([P, 512], mybir.dt.float32, tag="po")
                for kb in range(NKB):
                    nc.tensor.matmul(
                        po[:, :nch], lhsT=at_t[:, kb * P:(kb + 1) * P],
                        rhs=wsb[ci][:, kb, :],
                        start=(kb == 0), stop=(kb == NKB - 1))
                nc.vector.tensor_tensor(
                    out=o_sb[:, n0:n0 + nch], in0=po[:, :nch],
                    in1=bias_t[:, n0:n0 + nch], op=mybir.AluOpType.add)
            nc.sync.dma_start(out=OUT[mc * P:(mc + 1) * P, :], in_=o_sb)
        return

    # ---- A-chunk pipeline helpers ----
    def dma_a(mc):
        a_nat = apool.tile([P, K], mybir.dt.bfloat16)
        nc.gpsimd.dma_start(out=a_nat, in_=A[mc * P:(mc + 1) * P, :])  # casts f32->bf16
        return a_nat

    def tr_one(a_nat, at_tiles, kb):
        # element [p, b, t, i] = a_nat[p, 256b + 2i + t]; k-block 2b holds
        # k = 256b + 2p (low nibble), 2b+1 holds k = 256b + 2p + 1 (high).
        a_view = a_nat.rearrange("p (b i t) -> p b t i", b=NB2, i=P, t=2)
        b, par = kb // 2, kb % 2
        pt = psum_t.tile([P, P], mybir.dt.bfloat16, tag="pt")
        nc.tensor.transpose(pt, a_view[:, b, par, :], identity)
        at_t = atpool.tile([P, P], mybir.dt.bfloat16)
        nc.scalar.copy(out=at_t, in_=pt)
        at_tiles[kb] = at_t

    # ---- raw PE throughput probe: 6144 matmuls, minimal deps ----
    if variant == "mm_burst":
        at_fix = []
        for kb in range(NKB):
            at_t = atpool.tile([P, P], mybir.dt.bfloat16)
            nc.gpsimd.dma_start(out=at_t, in_=A[0:P, kb * P:(kb + 1) * P])
            at_fix.append(at_t)
        for mc in range(mch):
            for ci, (n0, nch) in enumerate(n_chunks):
                po = psum_o.tile([P, 512], mybir.dt.float32, tag="po")
                for kb in range(NKB):
                    nc.tensor.matmul(
                        po[:, :nch], lhsT=at_fix[kb], rhs=wsb[ci][:, kb, :],
                        start=(kb == 0), stop=(kb == NKB - 1))
                # cheap eviction straight to a reused SBUF tile (no DMA out)
                o_sb = opool.tile([P, 512], mybir.dt.float32, tag="ob")
                nc.vector.tensor_tensor(
                    out=o_sb[:, :nch], in0=po[:, :nch],
                    in1=bias_t[:, n0:n0 + nch], op=mybir.AluOpType.add)
        # token output write so OUT is bound
        nc.sync.dma_start(out=OUT[0:P, 0:512], in_=o_sb[:, :512])
        return

    # ---- timing-probe variants: at tiles filled once by cast-DMA, no
    # per-chunk transpose pipeline (numerically wrong; timing only) ----
    if variant in ("no_tr", "tr_no_copy", "tr_copy_no_dep", "half_mm", "one_w",
                   "no_adma", "f32_adma", "sync_adma", "no_out", "no_dq",
                   "lean_mm", "lean_half"):
        at_fix = []
        for kb in range(NKB):
            at_t = atpool.tile([P, P], mybir.dt.bfloat16)
            nc.gpsimd.dma_start(out=at_t, in_=A[0:P, kb * P:(kb + 1) * P])
            at_fix.append(at_t)
        def dma_a_probe(mc):
            if variant in ("no_adma", "lean_mm", "lean_half"):
                return None
            if variant == "f32_adma":
                a_nat = apool.tile([P, K], mybir.dt.float32, tag="a32")
                nc.gpsimd.dma_start(out=a_nat, in_=A[mc * P:(mc + 1) * P, :])
                return a_nat
            if variant == "sync_adma":
                a_nat = apool.tile([P, K], mybir.dt.float32, tag="a32")
                nc.sync.dma_start(out=a_nat, in_=A[mc * P:(mc + 1) * P, :])
                return a_nat
            return dma_a(mc)

        a_bufs = {}
        for mc in range(min(3, mch)):
            a_bufs[mc] = dma_a_probe(mc)
        for mc in range(mch):
            if mc + 3 < mch:
                a_bufs[mc + 3] = dma_a_probe(mc + 3)
            a_bufs.pop(mc - 1, None)
            do_tr = variant in ("tr_no_copy", "tr_copy_no_dep") and mc + 1 < mch
            if do_tr:
                a_view = a_bufs[mc + 1].rearrange(
                    "p (b i t) -> p b t i", b=NB2, i=P, t=2)
            o_sb = opool.tile([P, sum(c for _, c in n_chunks)], mybir.dt.float32)
            cnt = 0
            nkb_eff = NKB // 2 if variant in ("half_mm", "lean_half") else NKB
            for ci, (n0, nch) in enumerate(n_chunks):
                po = psum_o.tile([P, 512], mybir.dt.float32, tag="po")
                for kb in range(nkb_eff):
                    nc.tensor.matmul(
                        po[:, :nch],
                        lhsT=at_fix[0] if variant == "one_w" else at_fix[kb],
                        rhs=wsb[ci][:, kb, :],
                        start=(kb == 0), stop=(kb == nkb_eff - 1))
                    cnt += 1
                    if do_tr and cnt % 3 == 0:
                        kbt = cnt // 3 - 1
                        if kbt < NKB:
                            pt = psum_t.tile([P, P], mybir.dt.bfloat16, tag="pt")
                            nc.tensor.transpose(
                                pt, a_view[:, kbt // 2, kbt % 2, :], identity)
                            if variant == "tr_copy_no_dep":
                                sc_t = atpool.tile(
                                    [P, P], mybir.dt.bfloat16, tag="scratch")
                                nc.scalar.copy(out=sc_t, in_=pt)
                nc.vector.tensor_tensor(
                    out=o_sb[:, n0:n0 + nch], in0=po[:, :nch],
                    in1=bias_t[:, n0:n0 + nch], op=mybir.AluOpType.add)
            if variant not in ("no_out", "lean_mm", "lean_half"):
                nc.sync.dma_start(out=OUT[mc * P:(mc + 1) * P, :], in_=o_sb)
        if variant in ("no_out", "lean_mm", "lean_half"):
            nc.sync.dma_start(out=OUT[0:P, :], in_=o_sb)
        return

    # ---- prologue: chunk 0 (+1 prefetch) transposes overlap the dequant ----
    a_bufs = {0: dma_a(0)}
    if mch > 1:
        a_bufs[1] = dma_a(1)
    at_cur = [None] * NKB
    for kb in range(NKB):
        tr_one(a_bufs[0], at_cur, kb)

    # ---- main loop over 128-row chunks of A ----
    for mc in range(mch):
        if mc + 2 < mch:
            a_bufs[mc + 2] = dma_a(mc + 2)
        a_bufs.pop(mc - 1, None)

        at_next = [None] * NKB
        # interleave chunk-(mc+1) transposes among this chunk's matmuls:
        # one transpose+copy after every 3rd matmul (variant "full"), or
        # cluster them after the matmuls (variant "tail_tr").
        tr_pending = list(range(NKB)) if mc + 1 < mch else []
        interleave = variant == "full"

        o_sb = opool.tile([P, sum(c for _, c in n_chunks)], mybir.dt.float32)
        if variant == "kb_outer":
            # kb outer, n-chunk inner: one weight load serves 3 matmuls
            pos = []
            for _ci in range(len(n_chunks)):
                po_b = psum_o.tile([P, 512], mybir.dt.float32, tag="po")
                pos.append(po_b)
            for kb in range(NKB):
                for ci, (n0, nch) in enumerate(n_chunks):
                    nc.tensor.matmul(
                        pos[ci][:, :nch], lhsT=at_cur[kb], rhs=wsb[ci][:, kb, :],
                        start=(kb == 0), stop=(kb == NKB - 1))
            for ci, (n0, nch) in enumerate(n_chunks):
                nc.vector.tensor_tensor(
                    out=o_sb[:, n0:n0 + nch], in0=pos[ci][:, :nch],
                    in1=bias_t[:, n0:n0 + nch], op=mybir.AluOpType.add)
        else:
            cnt = 0
            for ci, (n0, nch) in enumerate(n_chunks):
                po = psum_o.tile([P, 512], mybir.dt.float32, tag="po")
                for kb in range(NKB):
                    nc.tensor.matmul(
                        po[:, :nch], lhsT=at_cur[kb], rhs=wsb[ci][:, kb, :],
                        start=(kb == 0), stop=(kb == NKB - 1))
                    cnt += 1
                    if interleave and cnt % 3 == 0 and tr_pending:
                        tr_one(a_bufs[mc + 1], at_next, tr_pending.pop(0))
                nc.vector.tensor_tensor(
                    out=o_sb[:, n0:n0 + nch], in0=po[:, :nch],
                    in1=bias_t[:, n0:n0 + nch], op=mybir.AluOpType.add)
        while tr_pending:
            tr_one(a_bufs[mc + 1], at_next, tr_pending.pop(0))
        nc.sync.dma_start(out=OUT[mc * P:(mc + 1) * P, :], in_=o_sb)
        at_cur = at_next


DEFAULT_VARIANT = "hostT8"

_NC_CACHE = {}


def _get_nc():
    if "nc" not in _NC_CACHE:
        _NC_CACHE["nc"] = build_nc(variant=DEFAULT_VARIANT)
    return _NC_CACHE["nc"]


def shard_inputs(A, weight, weight_scale, bias):
    A = np.ascontiguousarray(np.asarray(A, dtype=np.float32))
    wq = np.asarray(weight, dtype=np.uint8)
    ws = np.asarray(weight_scale, dtype=np.float32)
    bs = np.asarray(bias, dtype=np.float32)
    in_maps = []
    for c in range(NCORES):
        sl = slice(c * NS, (c + 1) * NS)
        in_maps.append({
            "A": A,
            "wq": np.ascontiguousarray(wq[:, sl]),
            # replicate each scale row 16x so row k2 of srep carries the
            # scale for packed row k2 (group g = k2 // 16)
            "srep": np.ascontiguousarray(np.repeat(ws[:, sl], 16, axis=0)),
            # partition-replicated so the device DMA is a plain 2D copy
            "bias": np.ascontiguousarray(np.broadcast_to(bs[sl], (P, NS))),
        })
    return in_maps


def host_transpose_a(A):
    """A^T tiles matching wsb's nibble-interleaved k-order:
    AT[mc*128 + p, (2b+t)*128 + j] = A[mc*128 + j, 256b + 2p + t]
    (k-block 2b holds k = 256b + 2p from the low nibble, 2b+1 the high),
    cast to bf16."""
    import ml_dtypes
    A = np.asarray(A, dtype=np.float32)
    m = A.shape[0]
    A5 = A.reshape(m // P, P, NB2, P, 2)        # [mc, j, b, p, t]
    At = A5.transpose(0, 3, 2, 4, 1)            # [mc, p, b, t, j]
    return np.ascontiguousarray(At.reshape(m, K).astype(ml_dtypes.bfloat16))


def shard_inputs_hostT(A, weight, weight_scale, bias):
    in_maps = shard_inputs(A, weight, weight_scale, bias)
    At = host_transpose_a(A)
    for m_ in in_maps:
        del m_["A"]
        m_["at"] = At
    return in_maps


def shard_inputs_timing(A, weight, weight_scale, bias, hostT=False):
    """Input maps for timing_io builds: A truncated to the small DRAM shape."""
    if hostT:
        in_maps = shard_inputs_hostT(A[:2 * P], weight, weight_scale, bias)
    else:
        in_maps = shard_inputs(A, weight, weight_scale, bias)
        for m_ in in_maps:
            m_["A"] = np.ascontiguousarray(m_["A"][:2 * P])
    return in_maps


def run(inputs, trace=False, **kw):
    nc = _get_nc()
    if DEFAULT_VARIANT.startswith("hostT"):
        in_maps = shard_inputs_hostT(**inputs)
    else:
        in_maps = shard_inputs(**inputs)
    res = run_bass_kernel_spmd(nc, in_maps, core_ids=list(range(NCORES)), trace=trace, **kw)
    out = np.concatenate([res.results[c]["out"] for c in range(NCORES)], axis=1)
    return out, res


def kernel(A, weight, weight_scale, bias):
    out, _ = run(dict(A=A, weight=weight, weight_scale=weight_scale, bias=bias))
    return out

